# revision 12
# baseline (speedup 1.0000x reference)
"""Trainium2 Bass kernel for nn_CFPT (cross-layer feature pyramid transformer block).

Per pyramid level l (feats [4, 256, H, W]) with halves f1 = feat[:2], f2 = feat[2:]:
    f1 = relu(f1 + swin_cross_attn(f1, f2, w1[l]))
    f2 = relu(f2 + swin_cross_attn(f2, f1, w2[l]))
Window attention (8x8 windows, 2 heads, dh=128) is window-local, and window w of one
half attends only to window w of the other half. Sharding by (image, window-row-block)
is fully data-parallel across the 8 cores with zero collectives: each core processes
1/8 of the windows of every level, both halves (sequentially, as the reference does).

The host reshapes activations to window-major layout [C, nWr, nWc, 64] (64 = tokens of
one 8x8 window) so every device-side matmul operand is a contiguous slice, and all
DMAs are contiguous. Device kernel (per core, SPMD, bf16 matmuls / fp32 PSUM):
    Q^T = WqT.T @ X^T (1/sqrt(dh) folded into WqT), K^T = WkT.T @ Y^T
    VW  = Y @ G per window pair, where G[c, 2*256] = per-head Wv_h.T @ Wo_h.T
          (the V and output projections fused into one matmul)
    S^T[k,q] = K @ Q^T per (window, head) into pair-stacked PSUM blocks
    P~ = exp(S^T) on ACT (scores are O(1): no max subtraction needed)
    Z = ones.T @ P~ (PE), R = 1/Z (DVE), row-broadcast R (gpsimd), P~ *= R (DVE)
    oproj^T = X^T (identity matmul, start=True) + sum_h VW_h.T @ P~_h (PSUM accum)
    out = relu(oproj^T) on ACT -> bf16 -> DMA (host casts back to fp32 + un-shuffles)
"""

import os
import sys
import numpy as np

for _p in ("/opt/trn_rl_repo",):
    if _p not in sys.path:
        sys.path.insert(0, _p)

import ml_dtypes
from contextlib import ExitStack

from concourse import bass, tile, mybir
from concourse import library_config
from concourse.bass_utils import run_bass_kernel_spmd

BF16 = mybir.dt.bfloat16
F32 = mybir.dt.float32
EXP = mybir.ActivationFunctionType.Exp
RELU = mybir.ActivationFunctionType.Relu
NPBF16 = ml_dtypes.bfloat16

N_CORES = 8
C = 256
DH = 128
WS = 8

# Per-core shard geometry per level. Core c: image bi = c // 4, quarter q = c % 4.
# Shard is [C, nWr, nWc, 64] window-major; superchunk = (wr, wc0, nwin).
LEVELS = [
    dict(nWr=4, nWc=16, scs=[(wr, wc0, 8) for wr in range(4) for wc0 in (0, 8)]),
    dict(nWr=2, nWc=8, scs=[(wr, 0, 8) for wr in range(2)]),
    dict(nWr=1, nWc=4, scs=[(0, 0, 4)]),
    dict(nWr=1, nWc=1, scs=[(0, 0, 1)]),
]

_BUILT = {}


def _patch_tile_drain():
    """The walrus in this container rejects instructions with >1 sync-wait
    ("Too many sync wait commands"). Two patches: (1) split every scheduled
    instruction's waits onto same-engine NOP carriers spliced in just before
    it; (2) split the kernel-tail Drain's waits across a chain of drains."""
    if getattr(tile.TileContext, "_drain_patched", False):
        return
    from concourse.tile import ScopedClock
    import bass_rust

    MAXW = 1
    orig_lower = tile.TileContext._lower_ordered_insts

    def _lower_ordered_insts(self, ordered):
        nc = self.nc
        for bb_name, insts in ordered.items():
            out = []
            for inst in insts:
                si = getattr(inst, "sync_info", None)
                if si is not None:
                    waits = list(si.on_wait)
                    if len(waits) > MAXW:
                        si.on_wait = waits[:MAXW]
                        rest = waits[MAXW:]
                        for i in range(0, len(rest), MAXW):
                            carrier = bass_rust.InstNoOp(
                                name=f"wsplit-{nc.next_id()}", ins=[], outs=[]
                            )
                            carrier.engine = inst.engine
                            carrier.sync_info = bass_rust.SyncInfo(
                                on_wait=rest[i : i + MAXW], on_update=[]
                            )
                            out.append(carrier)
                out.append(inst)
            insts[:] = out
        return orig_lower(self, ordered)

    tile.TileContext._lower_ordered_insts = _lower_ordered_insts

    def _drain_and_barrier(self, tick_clock, wait_clock):
        drain_inst = self.nc.sync.drain()
        wait_clock.add_sem_waits(
            drain_inst.ins, ScopedClock({None: tick_clock.global_clock})
        )
        si = drain_inst.ins.sync_info
        waits = list(si.on_wait) if si is not None else []
        if len(waits) > 1:
            si.on_wait = waits[:1]
            for w in waits[1:]:
                d2 = self.nc.sync.drain()
                d2.ins.sync_info = bass_rust.SyncInfo(on_wait=[w], on_update=[])
        self.nc.all_engine_barrier()
        assert self.sems is not None
        popped = self.nc._tile_sem_poison_stack.pop()
        assert popped is self._sem_poison
        self.nc.clear_and_free_semaphores(list(self.sems.allocated().values()))
        self.nc.all_engine_barrier()

    tile.TileContext._drain_and_barrier = _drain_and_barrier
    tile.TileContext._drain_patched = True


def _build_superchunk(nc, lvl, half, sc, X, Y, OUT, wq, wk, g, ident, ones, pools):
    """Emit one superchunk (<=8 windows, both heads) of one half-step."""
    wr, wc0, nwin = sc
    T = 64 * nwin
    npair = max(nwin // 2, 1)
    paired = nwin > 1
    kspan = 128 if paired else 64
    bw = 128 if paired else 64  # q-cols per pair block
    sb, pp, sps, zps, ops, dram_pool = pools
    uid = f"l{lvl}h{half}_{wr}_{wc0}"

    def view(t, cc):
        return t[cc][:, wr, wc0 : wc0 + nwin, :]  # [128, nwin, 64]

    # ---- Q^T / K^T projections ----
    qt, kt = [], []
    for which, src, w_t, dst in (("q", X, wq, qt), ("k", Y, wk, kt)):
        for dc in range(2):
            psq = pp.tile([128, T], F32, name=f"ps{which}_{uid}_{dc}", tag="proj")
            for cc in range(2):
                nc.tensor.matmul(
                    psq[:],
                    w_t[cc][:, dc * 128 : (dc + 1) * 128],
                    view(src, cc),
                    start=(cc == 0),
                    stop=(cc == 1),
                )
            t = sb.tile([128, nwin, 64], BF16, name=f"{which}t_{uid}_{dc}",
                        tag=f"{which}t{dc}", bufs=2)
            if which == "q":
                nc.vector.tensor_copy(t[:], psq[:])
            else:
                nc.scalar.copy(t[:], psq[:])
            dst.append(t)

    # ---- fused V*Wo projection: VW[pair tokens, 2*256] ----
    vw = []
    for p in range(npair):
        psv = pp.tile([128, 512], F32, name=f"psv_{uid}_{p}", tag="proj")
        for cc in range(2):
            lhs = view(Y, cc)[:, 2 * p : 2 * p + 2, :] if paired else view(Y, cc)
            nc.tensor.matmul(psv[:kspan, :], lhs, g[cc][:], start=(cc == 0), stop=(cc == 1))
        t = sb.tile([128, 512], BF16, name=f"vw_{uid}_{p}", tag=f"vw{p % 4}", bufs=2)
        if p % 2 == 0:
            nc.vector.tensor_copy(t[:kspan, :], psv[:kspan, :])
        else:
            nc.scalar.copy(t[:kspan, :], psv[:kspan, :])
        vw.append(t)

    # ---- scores S^T[k, q] per (head, pair); k/q both window-stacked ----
    spt = sps.tile([128, 2 * npair, bw], F32, name=f"spt_{uid}", tag="spt")
    for h in range(2):
        for p in range(npair):
            blk = h * npair + p
            rhs = qt[h][:, 2 * p : 2 * p + 2, :] if paired else qt[h][:, 0, :]
            nc.tensor.matmul(
                spt[0:64, blk], kt[h][:, 2 * p, :], rhs, start=True, stop=True,
            )
            if paired:
                nc.tensor.matmul(
                    spt[64:128, blk], kt[h][:, 2 * p + 1, :], rhs,
                    start=True, stop=True, tile_position=(0, 64),
                )

    # ---- softmax ----
    pt = sb.tile([128, 2 * npair, bw], BF16, name=f"pt_{uid}", tag="pt", bufs=2)
    nc.scalar.activation(pt[0:kspan], spt[0:kspan], EXP)
    if paired:
        # zero cross-window blocks so pair-stacked PV matmuls are exact
        nc.vector.memset(pt[0:64, :, 64:128], 0.0)
        nc.vector.memset(pt[64:128, :, 0:64], 0.0)

    zt = zps.tile([128, 512], F32, name=f"zt_{uid}", tag="zt")
    for h in range(2):
        nc.tensor.matmul(
            zt[h * 64 : h * 64 + 1, 0 : npair * bw],
            ones[0:kspan, :],
            pt[0:kspan, h * npair : (h + 1) * npair],
            start=True, stop=True,
        )
    rt = sb.tile([128, 512], BF16, name=f"rt_{uid}", tag="rt", bufs=2)
    with nc.allow_low_precision(reason="1/Z in bf16 matches the bf16 P~ it scales"):
        for h in range(2):
            nc.vector.reciprocal(
                rt[h * 64 : h * 64 + 1, 0 : npair * bw],
                zt[h * 64 : h * 64 + 1, 0 : npair * bw],
            )
    for h in range(2):
        # row-broadcast 1/Z along partitions via a DRAM round-trip (stride-0
        # partition reads are only legal from DRAM)
        rscr = dram_pool.tile([1, npair * bw], BF16, name=f"rs_{uid}_{h}", tag=f"rs{h}", bufs=2)
        nc.sync.dma_start(rscr[:], rt[h * 64 : h * 64 + 1, 0 : npair * bw])
        rb = sb.tile([128, npair, bw], BF16, name=f"rb_{uid}_{h}", tag=f"rb{h}", bufs=2)
        nc.sync.dma_start(rb[:], rscr[:].to_broadcast((128, npair * bw)))
        nc.vector.tensor_mul(
            pt[0:kspan, h * npair : (h + 1) * npair],
            pt[0:kspan, h * npair : (h + 1) * npair],
            rb[0:kspan],
        )

    # ---- residual + attention output + Wo, accumulated in PSUM ----
    # The identity (residual) matmul goes FIRST with start=True: start zeroes
    # the whole 2KB PSUM bank, so exactly one start per bank, and its full-tile
    # write orders it (WAW) before every accumulating PV matmul.
    for dc in range(2):
        opt_ = ops.tile([128, T], F32, name=f"op_{uid}_{dc}", tag=f"op{dc}")
        nc.tensor.matmul(
            opt_[:], ident[:], view(X, dc),
            start=True, stop=False, skip_group_check=True,
        )
        for p in range(npair):
            for h in range(2):
                nc.tensor.matmul(
                    opt_[:, p * bw : (p + 1) * bw],
                    vw[p][0:kspan, h * 256 + dc * 128 : h * 256 + dc * 128 + 128],
                    pt[0:kspan, h * npair + p],
                    start=False, stop=(p == npair - 1 and h == 1),
                    skip_group_check=True,
                )
        nc.scalar.activation(view(OUT, dc), opt_[:], RELU)


def _build_nc():
    _patch_tile_drain()
    nc = bass.Bass("TRN2", target_bir_lowering=False, debug=False)

    dins, douts = {}, {}
    for l, L in enumerate(LEVELS):
        shp = [C, L["nWr"], L["nWc"], 64]
        for t in ("a", "b"):
            dins[f"x{l}{t}"] = nc.dram_tensor(f"x{l}{t}", shp, BF16, kind="ExternalInput").ap()
            douts[f"y{l}{t}"] = nc.dram_tensor(f"y{l}{t}", shp, BF16, kind="ExternalOutput").ap()
        for d in range(2):
            dins[f"wq{l}{d}"] = nc.dram_tensor(f"wq{l}{d}", [C, C], BF16, kind="ExternalInput").ap()
            dins[f"wk{l}{d}"] = nc.dram_tensor(f"wk{l}{d}", [C, C], BF16, kind="ExternalInput").ap()
            dins[f"g{l}{d}"] = nc.dram_tensor(f"g{l}{d}", [C, 512], BF16, kind="ExternalInput").ap()
    dins["consts"] = nc.dram_tensor("consts", [128, 129], BF16, kind="ExternalInput").ap()

    with tile.TileContext(nc) as tc:
        with ExitStack() as ctx:
            wp = ctx.enter_context(tc.tile_pool(name="wp", bufs=1))
            xp = ctx.enter_context(tc.tile_pool(name="xp", bufs=1))
            sb = ctx.enter_context(tc.tile_pool(name="sb", bufs=2))
            pp = ctx.enter_context(tc.tile_pool(name="pp", bufs=3, space="PSUM"))
            sps = ctx.enter_context(tc.tile_pool(name="sps", bufs=1, space="PSUM"))
            zps = ctx.enter_context(tc.tile_pool(name="zps", bufs=1, space="PSUM"))
            ops = ctx.enter_context(tc.tile_pool(name="ops", bufs=1, space="PSUM"))
            dram_pool = ctx.enter_context(tc.tile_pool(name="drp", bufs=2, space="DRAM"))
            pools = (sb, pp, sps, zps, ops, dram_pool)

            const_t = wp.tile([128, 129], BF16, name="const_t")
            nc.sync.dma_start(const_t[:], dins["consts"][:])
            ident = const_t[:, 0:128]
            ones = const_t[:, 128:129]

            wts = {}
            for l in range(4):
                for d in range(2):
                    entry = []
                    for nm, wdt in (("wq", 256), ("wk", 256), ("g", 512)):
                        ccs = []
                        for cc in range(2):
                            t = wp.tile([128, wdt], BF16, name=f"{nm}_{l}_{d}_{cc}")
                            nc.sync.dma_start(t[:], dins[f"{nm}{l}{d}"][cc * 128 : (cc + 1) * 128, :])
                            ccs.append(t)
                        entry.append(ccs)
                    wts[(l, d)] = entry

            for l, L in enumerate(LEVELS):
                nWr, nWc = L["nWr"], L["nWc"]
                xa, xb, ya, yb = [
                    [xp.tile([128, nWr, nWc, 64], BF16, name=f"{nm}{l}_{cc}",
                             tag=f"{nm}{cc}", bufs=2 if nm in ("xa", "xb") else 1)
                     for cc in range(2)]
                    for nm in ("xa", "xb", "ya", "yb")
                ]
                for cc in range(2):
                    nc.sync.dma_start(xa[cc][:], dins[f"x{l}a"][cc * 128 : (cc + 1) * 128])
                    nc.sync.dma_start(xb[cc][:], dins[f"x{l}b"][cc * 128 : (cc + 1) * 128])
                for half in range(2):
                    Xh = xa if half == 0 else xb
                    Yh = xb if half == 0 else ya
                    Oh = ya if half == 0 else yb
                    wq, wk, g = wts[(l, half)]
                    for sc in L["scs"]:
                        _build_superchunk(nc, l, half, sc, Xh, Yh, Oh, wq, wk, g, ident, ones, pools)
                for cc in range(2):
                    nc.sync.dma_start(douts[f"y{l}a"][cc * 128 : (cc + 1) * 128], ya[cc][:])
                    nc.sync.dma_start(douts[f"y{l}b"][cc * 128 : (cc + 1) * 128], yb[cc][:])
    return nc


def get_nc():
    if "nc" not in _BUILT:
        _BUILT["nc"] = _build_nc()
    return _BUILT["nc"]


def _to_window_major(x):
    # [C, R, W] -> [C, R//8, W//8, 64]
    Cc, R, W = x.shape
    return np.ascontiguousarray(
        x.reshape(Cc, R // 8, 8, W // 8, 8).transpose(0, 1, 3, 2, 4).reshape(Cc, R // 8, W // 8, 64)
    )


def _from_window_major(x, R, W):
    Cc = x.shape[0]
    return x.reshape(Cc, R // 8, W // 8, 8, 8).transpose(0, 1, 3, 2, 4).reshape(Cc, R, W)


def _shard_slices(l, core):
    bi, q = core // 4, core % 4
    if l < 3:
        R = (32, 16, 8)[l]
        return bi, slice(q * R, (q + 1) * R), slice(None)
    return bi, slice((q // 2) * 8, (q // 2) * 8 + 8), slice((q % 2) * 8, (q % 2) * 8 + 8)


def make_inputs_for_core(core, feats, w1, w2):
    """Host-side shard + weight prep for one core. feats: np fp32 [4, C, H, W]."""
    m = {}
    for l in range(4):
        bi, hs, ws = _shard_slices(l, core)
        f = feats[l]
        m[f"x{l}a"] = _to_window_major(f[bi, :, hs, ws].astype(NPBF16))
        m[f"x{l}b"] = _to_window_major(f[2 + bi, :, hs, ws].astype(NPBF16))
    for l in range(4):
        for d, wsrc in enumerate((w1, w2)):
            Wq, Wk, Wv, Wo = [np.asarray(wsrc[l, j], np.float32) for j in range(4)]
            m[f"wq{l}{d}"] = np.ascontiguousarray((Wq.T * DH ** -0.5).astype(NPBF16))
            m[f"wk{l}{d}"] = np.ascontiguousarray(Wk.T.astype(NPBF16))
            gs = [Wv[h * 128 : (h + 1) * 128, :].T @ Wo[:, h * 128 : (h + 1) * 128].T
                  for h in range(2)]
            m[f"g{l}{d}"] = np.ascontiguousarray(np.concatenate(gs, 1).astype(NPBF16))
    consts = np.zeros((128, 129), np.float32)
    consts[:, :128] = np.eye(128)
    consts[:, 128] = 1.0
    m["consts"] = consts.astype(NPBF16)
    return m


def assemble_outputs(results, shapes):
    outs = []
    for l in range(4):
        full = np.zeros(shapes[l], np.float32)
        R, W = shapes[l][2], shapes[l][3]
        for core in range(N_CORES):
            bi, hs, ws = _shard_slices(l, core)
            Rs = hs.stop - hs.start if hs.start is not None else R
            Ws = (ws.stop - ws.start) if ws != slice(None) else W
            ya = _from_window_major(results[core][f"y{l}a"].astype(np.float32), Rs, Ws)
            yb = _from_window_major(results[core][f"y{l}b"].astype(np.float32), Rs, Ws)
            full[bi, :, hs, ws] = ya
            full[2 + bi, :, hs, ws] = yb
        outs.append(full)
    return tuple(outs)


def kernel(feat0, feat1, feat2, feat3, w1, w2):
    feats = [np.asarray(f, np.float32) for f in (feat0, feat1, feat2, feat3)]
    w1 = np.asarray(w1, np.float32)
    w2 = np.asarray(w2, np.float32)
    nc = get_nc()
    in_maps = [make_inputs_for_core(c, feats, w1, w2) for c in range(N_CORES)]
    res = run_bass_kernel_spmd(nc, in_maps, list(range(N_CORES)))
    return assemble_outputs(res.results, [f.shape for f in feats])
